# revision 1
# baseline (speedup 1.0000x reference)
"""Sparse-attention Trainium2 kernel (nn_Attention_44341242364527).

Strategy
--------
Head-tensor-parallel over 8 NeuronCores (2 heads/core, Megatron-style:
Wq/Wk/Wv column-sharded, Wo row-sharded, partial outputs all-reduced on
the host during unshard).

The sparse gather ``k[idx]`` / ``v[idx]`` is reformulated densely: since
``exp(qk/sqrt(D) + geo) = exp(qk/sqrt(D)) * exp(geo)``, and idx/valid/
geo_bias are host-known inputs, the host pre-scatters

    WT[h][s', s] = sum_k 1[idx[s,k]==s' & valid & s'<=s] * exp(geo[h,s,k])

Then per head, on device (everything transposed so no on-chip transposes
are needed):

    ST  = Kh @ Qh.T                  [s', s]   (dense scores)
    AT  = exp(ST/sqrt(D)) * WT       [s', s]   (un-normalized attention)
    AOT = Vh.T @ AT                  [d, s]    (un-normalized context)
    Z   = ones @ AT                  [1, s]    (softmax denominator)
    Y  += (AOT/Z).T @ WoT_shard      [s, HID]  (partial output)

Causality makes AT block-lower-triangular: only ~62% of blocks are
computed. WT==0 kills both the masked and the un-selected entries.

Precision: PSUM accumulation is fp32 everywhere. Matmul operands are
float32r (1 cycle/row on the PE vs 4 for fp32; ~3e-4 rel err) except the
QKV projection inputs, the scattered WT, and the partial output y, which
are bf16 (DMA-bound tensors). End-to-end rel err vs the fp32 reference
is ~3.4e-3. The three stages (projection, attention, output projection)
are emitted chunk-interleaved so the Tile scheduler pipelines them; the
attention AV/Z matmuls trail the ST/exp/mul stage by two tiles to hide
the PE->ACT->DVE->PE latency chain.
"""

import math
import sys

sys.path.insert(0, "/opt/trn_rl_repo")

import numpy as np

B, S, H, D, KS = 1, 2048, 16, 128, 64
HID = H * D
NCORES = 8
HPC = H // NCORES          # heads per core
CPC = HPC * D              # output channels per core
P = 128                    # partitions
SC = 512                   # s-chunk (PSUM bank width in f32)
NJ = S // SC               # 4 s-chunks
NT = S // P                # 16 s'-tiles
NK = HID // P              # 16 contraction chunks

# dtype knobs (numpy dtype name per tensor class); PSUM is always f32.
DT_PROJ = "bfloat16"       # hsT + Wq/Wk/Wv operands of the QKV projections
DT_QK = "float32r"         # Q^T/K^T operands of the score matmul
DT_ATT = "float32r"        # exp(S)*W and V operands of the AV matmul
DT_WT = "bfloat16"         # the scattered exp(geo) tensor (DMA-heavy)
DT_WO = "float32r"         # AOT and Wo operands of the output projection

_CACHE = {}


def _np_dt(name):
    if name == "bfloat16":
        import ml_dtypes

        return np.dtype(ml_dtypes.bfloat16)
    if name == "float32r":
        return np.dtype(np.float32)
    return np.dtype(name)


def _my_dt(name):
    from concourse import mybir

    return {
        "float32": mybir.dt.float32,
        "float32r": mybir.dt.float32r,
        "bfloat16": mybir.dt.bfloat16,
    }[name]


def _build_nc(reps=1):
    import concourse.tile as tile
    from concourse import bacc, mybir

    F32 = mybir.dt.float32
    EXP = mybir.ActivationFunctionType.Exp
    MULT = mybir.AluOpType.mult

    nc = bacc.Bacc("TRN2", target_bir_lowering=False, debug=False,
                   num_devices=NCORES)

    hsT = nc.dram_tensor("hsT", [HID, S], _my_dt(DT_PROJ), kind="ExternalInput")
    wqT = nc.dram_tensor("wqT", [HID, CPC], _my_dt(DT_PROJ), kind="ExternalInput")
    wkT = nc.dram_tensor("wkT", [HID, CPC], _my_dt(DT_PROJ), kind="ExternalInput")
    wvT = nc.dram_tensor("wvT", [HID, CPC], _my_dt(DT_PROJ), kind="ExternalInput")
    woT = nc.dram_tensor("woT", [CPC, HID], _my_dt(DT_WO), kind="ExternalInput")
    wt = nc.dram_tensor("wt", [HPC, S, S], _my_dt(DT_WT), kind="ExternalInput")
    y = nc.dram_tensor("y", [S, HID], mybir.dt.bfloat16, kind="ExternalOutput")

    inv_sqrt_d = 1.0 / math.sqrt(D)
    F32R = mybir.dt.float32r

    def mm(out, lhsT, rhs, **kw):
        nc.tensor.matmul(out, lhsT, rhs, **kw)

    with tile.TileContext(nc) as tc, \
            nc.allow_low_precision(reason="fp32r matmul operands; PSUM accum stays f32"):
        with tc.tile_pool(name="persist", bufs=1) as persist:
            QT = [persist.tile([P, S], _my_dt(DT_QK), tag=f"qt{h}", name=f"qt{h}")
                  for h in range(HPC)]
            KT = [persist.tile([P, S], _my_dt(DT_QK), tag=f"kt{h}", name=f"kt{h}")
                  for h in range(HPC)]
            Vsb = [persist.tile([P, CPC], _my_dt(DT_ATT), tag=f"v{t}", name=f"vres{t}")
                   for t in range(NT)]
            AOT = [persist.tile([P, S], _my_dt(DT_WO), tag=f"aot{h}", name=f"aot{h}")
                   for h in range(HPC)]
            ones_col = persist.tile([P, 1], _my_dt(DT_ATT), tag="ones_col", name="ones_col")
            ones_row = persist.tile([1, P], F32R, tag="ones_row", name="ones_row")
            ones_f32 = persist.tile([P, 1], F32, tag="ones_f32", name="ones_f32")
            onesr_f32 = persist.tile([1, P], F32, tag="onesr_f32", name="onesr_f32")
            nc.gpsimd.memset(ones_f32[:], 1.0)
            nc.gpsimd.memset(onesr_f32[:], 1.0)
            nc.vector.tensor_copy(ones_col[:], ones_f32[:])
            nc.vector.tensor_copy(ones_row[:], onesr_f32[:])

            # Pipelined over j-chunks of the query/sequence dim: for each j,
            # project chunk j, run attention for chunk j (both heads), then
            # the output projection for s-tiles 4j..4j+3. Chunk-level deps
            # let the Tile scheduler overlap all three stages across j.
            with tc.tile_pool(name="wpool", bufs=1) as wpool, \
                 tc.tile_pool(name="hpool", bufs=20) as hpool, \
                 tc.tile_pool(name="wop", bufs=1) as wop, \
                 tc.tile_pool(name="wtp", bufs=5) as wtp, \
                 tc.tile_pool(name="atp", bufs=4) as atp, \
                 tc.tile_pool(name="rbp", bufs=2) as rbp, \
                 tc.tile_pool(name="smp", bufs=2) as smp, \
                 tc.tile_pool(name="ypool", bufs=4) as ypool, \
                 tc.tile_pool(name="psX", bufs=3, space="PSUM") as psX, \
                 tc.tile_pool(name="psA", bufs=2, space="PSUM") as psA, \
                 tc.tile_pool(name="psZ", bufs=2, space="PSUM") as psZ, \
                 tc.tile_pool(name="psY", bufs=1, space="PSUM") as psY:
                wq_sb, wk_sb, wv_sb = [], [], []
                wo_sb = []

                for _rep in range(reps):
                    for j in range(NJ):
                        # -- QKV projection for chunk j --
                        # (weight loads interleaved k-wise with the first
                        # chunk's hsT loads so PE starts ~1us in, not after
                        # 8MB of weight DMA)
                        hs_t = []
                        for k in range(NK):
                            if _rep == 0 and j == 0:
                                for lst, dram, nm in ((wq_sb, wqT, "wq"),
                                                      (wk_sb, wkT, "wk"),
                                                      (wv_sb, wvT, "wv")):
                                    t_ = wpool.tile([P, CPC], _my_dt(DT_PROJ),
                                                    tag=f"{nm}{k}", name=f"{nm}{k}")
                                    nc.sync.dma_start(
                                        t_[:], dram[k * P:(k + 1) * P, :])
                                    lst.append(t_)
                            t_ = hpool.tile([P, SC], _my_dt(DT_PROJ), tag="hst", name="hst")
                            nc.sync.dma_start(
                                t_[:], hsT[k * P:(k + 1) * P, j * SC:(j + 1) * SC])
                            hs_t.append(t_)
                        if _rep == 0 and j == 0:
                            for h in range(HPC):
                                t_ = wop.tile([P, HID], _my_dt(DT_WO),
                                              tag=f"wo{h}", name=f"wo{h}")
                                nc.sync.dma_start(t_[:], woT[h * P:(h + 1) * P, :])
                                wo_sb.append(t_)
                        for h in range(HPC):
                            for w_sb, acc in ((wq_sb, QT), (wk_sb, KT)):
                                pp = psX.tile([P, SC], F32, tag="big", name="ps_proj")
                                for k in range(NK):
                                    mm(pp[:], w_sb[k][:, h * D:(h + 1) * D],
                                       hs_t[k][:],
                                       start=(k == 0), stop=(k == NK - 1))
                                nc.vector.tensor_copy(
                                    acc[h][:, j * SC:(j + 1) * SC], pp[:])
                        for si in range(SC // P):
                            vp = psX.tile([P, CPC], F32, tag="big", name="ps_projv")
                            for k in range(NK):
                                mm(vp[:], hs_t[k][:, si * P:(si + 1) * P],
                                   wv_sb[k][:],
                                   start=(k == 0), stop=(k == NK - 1))
                            nc.vector.tensor_copy(Vsb[4 * j + si][:], vp[:])

                        # -- attention for chunk j: both heads interleaved,
                        # AV/Z matmuls lag the ST/exp/mul stage by 2 items so
                        # the PE never waits on the ACT->DVE latency chain --
                        tmax = min(4 * j + 3, NT - 1)
                        aop = [psA.tile([P, SC], F32, tag="ao", name=f"ao{h}")
                               for h in range(HPC)]
                        zp = [psZ.tile([1, SC], F32, tag="z", name=f"z{h}")
                              for h in range(HPC)]
                        items = [(t, h) for t in range(tmax + 1)
                                 for h in range(HPC)]
                        pend = []

                        def drain_one():
                            t_, h_, at_, o_, w_ = pend.pop(0)
                            mm(aop[h_][:, o_:SC],
                               Vsb[t_][:, h_ * D:(h_ + 1) * D],
                               at_[:, :w_],
                               start=(t_ == 0), stop=(t_ == tmax))
                            mm(zp[h_][:, o_:SC], ones_col[:], at_[:, :w_],
                               start=(t_ == 0), stop=(t_ == tmax))

                        for t, h in items:
                            # within the diagonal block only columns
                            # s >= 128t are causally reachable (WT is zero
                            # elsewhere) -- shrink every stage to that width
                            o = max(0, t * P - j * SC)
                            w = SC - o
                            stp = psX.tile([P, SC], F32, tag="big", name="st")
                            mm(stp[:, :w], KT[h][:, t * P:(t + 1) * P],
                               QT[h][:, j * SC + o:(j + 1) * SC],
                               start=True, stop=True)
                            at = atp.tile([P, SC], _my_dt(DT_ATT), tag="at", name="at")
                            nc.scalar.activation(at[:, :w], stp[:, :w], EXP,
                                                 scale=inv_sqrt_d)
                            wt_sb = wtp.tile([P, SC], _my_dt(DT_WT), tag="wt", name="wt")
                            nc.sync.dma_start(
                                wt_sb[:, :w],
                                wt[h, t * P:(t + 1) * P,
                                   j * SC + o:(j + 1) * SC])
                            nc.vector.tensor_mul(at[:, :w], at[:, :w],
                                                 wt_sb[:, :w])
                            pend.append((t, h, at, o, w))
                            if len(pend) >= 3:
                                drain_one()
                        while pend:
                            drain_one()

                        for h in range(HPC):
                            r = smp.tile([1, SC], F32R, tag="r", name="r")
                            nc.vector.reciprocal(r[:], zp[h][:])
                            rb = psX.tile([P, SC], F32, tag="big", name="rb")
                            mm(rb[:], ones_row[:], r[:],
                               start=True, stop=True)
                            rbs = rbp.tile([P, SC], F32, tag="rbs", name="rbs")
                            nc.scalar.copy(rbs[:], rb[:])
                            nc.vector.tensor_tensor(
                                AOT[h][:, j * SC:(j + 1) * SC], aop[h][:],
                                rbs[:], MULT)

                        # -- output projection for s-tiles of chunk j --
                        for m in range(4 * j, 4 * j + 4):
                            for n in range(NJ):
                                yps = psY.tile([P, SC], F32, tag="y", name="ps_y")
                                for h in range(HPC):
                                    mm(yps[:], AOT[h][:, m * P:(m + 1) * P],
                                       wo_sb[h][:, n * SC:(n + 1) * SC],
                                       start=(h == 0), stop=(h == HPC - 1))
                                ysb = ypool.tile([P, SC], mybir.dt.bfloat16,
                                                 tag="ysb", name="ysb")
                                nc.vector.tensor_copy(ysb[:], yps[:])
                                nc.sync.dma_start(
                                    y[m * P:(m + 1) * P, n * SC:(n + 1) * SC],
                                    ysb[:])

    nc.compile()
    return nc


def _get_nc():
    if "nc" not in _CACHE:
        _CACHE["nc"] = _build_nc()
    return _CACHE["nc"]


def make_in_maps(hidden_states, idx, valid, geo_bias, Wq, Wk, Wv, Wo):
    """Host-side sharding/layout prep: returns the 8 per-core input maps."""
    hs = np.ascontiguousarray(np.asarray(hidden_states, np.float32)[0])
    idx = np.asarray(idx).astype(np.int64)
    valid = np.asarray(valid).astype(bool)
    geo = np.asarray(geo_bias, np.float32)

    dt_proj, dt_wo, dt_wt = _np_dt(DT_PROJ), _np_dt(DT_WO), _np_dt(DT_WT)

    hsT = np.ascontiguousarray(hs.T).astype(dt_proj)       # [HID, S]

    srange = np.arange(S)
    cmask = ((idx <= srange[:, None]) & valid).ravel()
    flat = (idx * S + srange[:, None]).ravel()[cmask]
    eg = np.exp(np.asarray(geo_bias, np.float64))          # [H, S, K]

    in_maps = []
    for c in range(NCORES):
        h0 = HPC * c
        sl = slice(h0 * D, (h0 + HPC) * D)
        wt_c = np.empty((HPC, S, S), dt_wt)
        for hh in range(HPC):
            wt_c[hh] = (np.bincount(flat,
                                    weights=eg[h0 + hh].ravel()[cmask],
                                    minlength=S * S)
                        .reshape(S, S).astype(dt_wt))
        in_maps.append({
            "hsT": hsT,
            "wqT": np.ascontiguousarray(np.asarray(Wq)[sl].T).astype(dt_proj),
            "wkT": np.ascontiguousarray(np.asarray(Wk)[sl].T).astype(dt_proj),
            "wvT": np.ascontiguousarray(np.asarray(Wv)[sl].T).astype(dt_proj),
            "woT": np.ascontiguousarray(np.asarray(Wo)[:, sl].T).astype(dt_wo),
            "wt": wt_c,
        })
    return in_maps


def kernel(hidden_states, idx, valid, geo_bias, Wq, Wk, Wv, Wo, bo):
    from concourse import bass_utils

    nc = _get_nc()
    in_maps = make_in_maps(hidden_states, idx, valid, geo_bias, Wq, Wk, Wv, Wo)
    res = bass_utils.run_bass_kernel_spmd(nc, in_maps,
                                          core_ids=list(range(NCORES)))
    out = np.zeros((S, HID), np.float32)
    for r in res.results:
        out += r["y"].astype(np.float32)
    out += np.asarray(bo, np.float32)
    return out.reshape(B, S, HID)



# revision 12
# speedup vs baseline: 1.0721x; 1.0721x over previous
"""Sparse-attention Trainium2 kernel (nn_Attention_44341242364527).

Strategy
--------
Head-tensor-parallel over 8 NeuronCores (2 heads/core, Megatron-style:
Wq/Wk/Wv column-sharded, Wo row-sharded, partial outputs all-reduced on
the host during unshard).

The sparse gather ``k[idx]`` / ``v[idx]`` is reformulated densely: since
``exp(qk/sqrt(D) + geo) = exp(qk/sqrt(D)) * exp(geo)``, and idx/valid/
geo_bias are host-known inputs, the host pre-scatters

    WT[h][s', s] = sum_k 1[idx[s,k]==s' & valid & s'<=s] * exp(geo[h,s,k])

Then per head, on device (everything transposed so no on-chip transposes
are needed):

    ST  = Kh @ Qh.T                  [s', s]   (dense scores)
    AT  = exp(ST/sqrt(D)) * WT       [s', s]   (un-normalized attention)
    AOT = Vh.T @ AT                  [d, s]    (un-normalized context)
    Z   = colsum(AT)                 [1, s]    (softmax denominator)
    Y  += (AOT/Z).T @ WoT_shard      [s, HID]  (partial output)

Causality makes AT block-lower-triangular: only ~62% of blocks are
computed. WT==0 kills both the masked and the un-selected entries.

v2 changes vs the 193us baseline:
  - Z (the column sum of AT over s') moved off the PE: the AT tiles are
    tree-accumulated on the DVE (f32, SBUF 2x mode) into acc[128,512],
    then ONE GPSIMD partition_all_reduce per (j,head) broadcasts the
    128-partition sum to every partition. The broadcast output also
    replaces the ones-outer-product matmul (and its ACT copy) that used
    to materialize 1/Z across partitions: reciprocal runs directly on
    the broadcast. Saves ~39k PE cycles (~16us) per rep.
  - All matmul operands bf16 (was float32r): kills the fp32r 4x
    cycles/row penalty on <256-wide diagonal tiles, enables FWL on
    stationary loads and 2x/4x DVE modes on the exp*WT multiply.
  - PSUM rebalance: freed Z banks -> psX 4 bufs, psY 2 bufs (output
    projection double-buffered).
  - DMA batching: WT loaded in [128, 4*512] groups via a strided
    (t p) c -> p t c access pattern (20 descriptors/rep instead of 80);
    y written one row-block [128, 2048] at a time (16 instead of 64).
"""

import math
import sys

sys.path.insert(0, "/opt/trn_rl_repo")

import numpy as np

B, S, H, D, KS = 1, 2048, 16, 128, 64
HID = H * D
NCORES = 8
HPC = H // NCORES          # heads per core
CPC = HPC * D              # output channels per core
P = 128                    # partitions
SC = 512                   # s-chunk (PSUM bank width in f32)
NJ = S // SC               # 4 s-chunks
NT = S // P                # 16 s'-tiles
NK = HID // P              # 16 contraction chunks

# dtype knobs (numpy dtype name per tensor class); PSUM is always f32.
DT_PROJ = "bfloat16"       # hsT + Wq/Wk/Wv operands of the QKV projections
DT_QK = "bfloat16"         # Q^T/K^T operands of the score matmul
DT_ATT = "bfloat16"        # exp(S)*W and V operands of the AV matmul
DT_WT = "bfloat16"         # the scattered exp(geo) tensor (DMA-heavy)
DT_WO = "bfloat16"         # AOT and Wo operands of the output projection

_CACHE = {}


def _np_dt(name):
    if name == "bfloat16":
        import ml_dtypes

        return np.dtype(ml_dtypes.bfloat16)
    if name == "float32r":
        return np.dtype(np.float32)
    return np.dtype(name)


def _my_dt(name):
    from concourse import mybir

    return {
        "float32": mybir.dt.float32,
        "float32r": mybir.dt.float32r,
        "bfloat16": mybir.dt.bfloat16,
    }[name]


def _build_nc(reps=1):
    import concourse.tile as tile
    from concourse import bacc, bass_isa, mybir

    F32 = mybir.dt.float32
    EXP = mybir.ActivationFunctionType.Exp
    MULT = mybir.AluOpType.mult
    ADD = mybir.AluOpType.add
    RADD = bass_isa.ReduceOp.add

    nc = bacc.Bacc("TRN2", target_bir_lowering=False, debug=False,
                   num_devices=NCORES)

    hsT = nc.dram_tensor("hsT", [HID, S], _my_dt(DT_PROJ), kind="ExternalInput")
    wqT = nc.dram_tensor("wqT", [HID, CPC], _my_dt(DT_PROJ), kind="ExternalInput")
    wkT = nc.dram_tensor("wkT", [HID, CPC], _my_dt(DT_PROJ), kind="ExternalInput")
    wvT = nc.dram_tensor("wvT", [HID, CPC], _my_dt(DT_PROJ), kind="ExternalInput")
    woT = nc.dram_tensor("woT", [CPC, HID], _my_dt(DT_WO), kind="ExternalInput")
    wt = nc.dram_tensor("wt", [HPC, S, S], _my_dt(DT_WT), kind="ExternalInput")
    y = nc.dram_tensor("y", [S, HID], mybir.dt.bfloat16, kind="ExternalOutput")

    inv_sqrt_d = 1.0 / math.sqrt(D)

    def mm(out, lhsT, rhs, **kw):
        nc.tensor.matmul(out, lhsT, rhs, **kw)

    with tile.TileContext(nc) as tc, \
            nc.allow_low_precision(reason="bf16 matmul operands; PSUM accum stays f32"):
        with tc.tile_pool(name="persist", bufs=1) as persist:
            QT = [persist.tile([P, S], _my_dt(DT_QK), tag=f"qt{h}", name=f"qt{h}")
                  for h in range(HPC)]
            KT = [persist.tile([P, S], _my_dt(DT_QK), tag=f"kt{h}", name=f"kt{h}")
                  for h in range(HPC)]
            Vsb = [persist.tile([P, CPC], _my_dt(DT_ATT), tag=f"v{t}", name=f"vres{t}")
                   for t in range(NT)]
            AOT = [persist.tile([P, S], _my_dt(DT_WO), tag=f"aot{h}", name=f"aot{h}")
                   for h in range(HPC)]

            # Pipelined over j-chunks of the query/sequence dim: for each j,
            # project chunk j, run attention for chunk j (both heads), then
            # the output projection for s-tiles 4j..4j+3. Chunk-level deps
            # let the Tile scheduler overlap all three stages across j.
            with tc.tile_pool(name="wpool", bufs=1) as wpool, \
                 tc.tile_pool(name="hpool", bufs=20) as hpool, \
                 tc.tile_pool(name="wop", bufs=1) as wop, \
                 tc.tile_pool(name="wtp", bufs=5) as wtp, \
                 tc.tile_pool(name="atp", bufs=5) as atp, \
                 tc.tile_pool(name="rbp", bufs=4) as rbp, \
                 tc.tile_pool(name="zacc", bufs=4) as zaccp, \
                 tc.tile_pool(name="ypool", bufs=3) as ypool, \
                 tc.tile_pool(name="psP", bufs=2, space="PSUM") as psP, \
                 tc.tile_pool(name="psS", bufs=2, space="PSUM") as psS, \
                 tc.tile_pool(name="psA", bufs=2, space="PSUM") as psA, \
                 tc.tile_pool(name="psY", bufs=2, space="PSUM") as psY:
                wq_sb, wk_sb, wv_sb = [], [], []
                wo_sb = []

                for _rep in range(reps):
                    for j in range(NJ):
                        # -- QKV projection for chunk j --
                        # (weight loads interleaved k-wise with the first
                        # chunk's hsT loads so PE starts ~1us in, not after
                        # the full weight DMA)
                        hs_t = []
                        for k in range(NK):
                            if _rep == 0 and j == 0:
                                for lst, dram, nm in ((wq_sb, wqT, "wq"),
                                                      (wk_sb, wkT, "wk"),
                                                      (wv_sb, wvT, "wv")):
                                    t_ = wpool.tile([P, CPC], _my_dt(DT_PROJ),
                                                    tag=f"{nm}{k}", name=f"{nm}{k}")
                                    nc.sync.dma_start(
                                        t_[:], dram[k * P:(k + 1) * P, :])
                                    lst.append(t_)
                            t_ = hpool.tile([P, SC], _my_dt(DT_PROJ), tag="hst", name="hst")
                            nc.sync.dma_start(
                                t_[:], hsT[k * P:(k + 1) * P, j * SC:(j + 1) * SC])
                            hs_t.append(t_)
                        if _rep == 0 and j == 0:
                            for h in range(HPC):
                                t_ = wop.tile([P, HID], _my_dt(DT_WO),
                                              tag=f"wo{h}", name=f"wo{h}")
                                nc.sync.dma_start(t_[:], woT[h * P:(h + 1) * P, :])
                                wo_sb.append(t_)
                        for h in range(HPC):
                            for w_sb, acc in ((wq_sb, QT), (wk_sb, KT)):
                                pp = psP.tile([P, SC], F32, tag="big", name="ps_proj")
                                for k in range(NK):
                                    mm(pp[:], w_sb[k][:, h * D:(h + 1) * D],
                                       hs_t[k][:],
                                       start=(k == 0), stop=(k == NK - 1))
                                nc.scalar.copy(
                                    acc[h][:, j * SC:(j + 1) * SC], pp[:])
                        for si in range(SC // P):
                            vp = psP.tile([P, CPC], F32, tag="big", name="ps_projv")
                            for k in range(NK):
                                mm(vp[:], hs_t[k][:, si * P:(si + 1) * P],
                                   wv_sb[k][:],
                                   start=(k == 0), stop=(k == NK - 1))
                            nc.vector.tensor_copy(Vsb[4 * j + si][:], vp[:])

                        # -- attention for chunk j: both heads interleaved,
                        # AV matmuls lag the ST/exp/mul stage by 2 items so
                        # the PE never waits on the ACT->DVE latency chain.
                        # Z runs off-PE: DVE tree-accumulates the AT tiles
                        # into acc[h], then one GPSIMD partition_all_reduce
                        # broadcasts the column sum to all partitions. --
                        tmax = min(4 * j + 3, NT - 1)
                        aop = [psA.tile([P, SC], F32, tag="ao", name=f"ao{h}")
                               for h in range(HPC)]
                        zacc = [zaccp.tile([P, SC], F32, tag="za", name=f"za{h}")
                                for h in range(HPC)]
                        wtg = {}
                        for h in range(HPC):
                            for g in range(0, tmax + 1, 4):
                                t_ = wtp.tile([P, 4 * SC], _my_dt(DT_WT),
                                              tag="wt", name="wt")
                                dram_v = wt[h].rearrange(
                                    "(t p) c -> p t c", p=P)[
                                        :, g:g + 4, j * SC:(j + 1) * SC]
                                sb_v = t_[:].rearrange("p (t c) -> p t c", c=SC)
                                nc.sync.dma_start(sb_v, dram_v)
                                wtg[(h, g)] = t_
                        items = [(t, h) for t in range(tmax + 1)
                                 for h in range(HPC)]
                        pend = []

                        def drain_one():
                            t_, h_, at_, o_, w_ = pend.pop(0)
                            mm(aop[h_][:, o_:SC],
                               Vsb[t_][:, h_ * D:(h_ + 1) * D],
                               at_[:, :w_],
                               start=(t_ == 0), stop=(t_ == tmax))

                        for t, h in items:
                            # within the diagonal block only columns
                            # s >= 128t are causally reachable (WT is zero
                            # elsewhere) -- shrink every stage to that width
                            o = max(0, t * P - j * SC)
                            w = SC - o
                            stp = psS.tile([P, SC], F32, tag="st", name="st")
                            mm(stp[:, :w], KT[h][:, t * P:(t + 1) * P],
                               QT[h][:, j * SC + o:(j + 1) * SC],
                               start=True, stop=True)
                            at = atp.tile([P, SC], _my_dt(DT_ATT), tag="at", name="at")
                            nc.scalar.activation(at[:, :w], stp[:, :w], EXP,
                                                 scale=inv_sqrt_d)
                            g = (t // 4) * 4
                            base = (t - g) * SC
                            nc.vector.tensor_mul(at[:, :w], at[:, :w],
                                                 wtg[(h, g)][:, base + o:base + SC])
                            if t == 0:
                                nc.vector.tensor_copy(zacc[h][:], at[:])
                            else:
                                nc.vector.tensor_tensor(
                                    zacc[h][:, o:SC], zacc[h][:, o:SC],
                                    at[:, :w], ADD)
                            pend.append((t, h, at, o, w))
                            if len(pend) >= 3:
                                drain_one()
                        while pend:
                            drain_one()

                        for h in range(HPC):
                            zb = rbp.tile([P, SC], F32, tag="zb", name="zb")
                            nc.gpsimd.partition_all_reduce(
                                zb[:], zacc[h][:], channels=P, reduce_op=RADD)
                            rbs = rbp.tile([P, SC], F32, tag="rbs", name="rbs")
                            nc.vector.reciprocal(rbs[:], zb[:])
                            nc.vector.tensor_tensor(
                                AOT[h][:, j * SC:(j + 1) * SC], aop[h][:],
                                rbs[:], MULT)

                        # -- output projection for s-tiles of chunk j --
                        for m in range(4 * j, 4 * j + 4):
                            ysb = ypool.tile([P, S], mybir.dt.bfloat16,
                                             tag="ysb", name="ysb")
                            for n in range(NJ):
                                yps = psY.tile([P, SC], F32, tag="y", name="ps_y")
                                for h in range(HPC):
                                    mm(yps[:], AOT[h][:, m * P:(m + 1) * P],
                                       wo_sb[h][:, n * SC:(n + 1) * SC],
                                       start=(h == 0), stop=(h == HPC - 1))
                                nc.scalar.copy(
                                    ysb[:, n * SC:(n + 1) * SC], yps[:])
                            nc.sync.dma_start(y[m * P:(m + 1) * P, :], ysb[:])

    nc.compile()
    return nc


def _get_nc():
    if "nc" not in _CACHE:
        _CACHE["nc"] = _build_nc()
    return _CACHE["nc"]


def make_in_maps(hidden_states, idx, valid, geo_bias, Wq, Wk, Wv, Wo):
    """Host-side sharding/layout prep: returns the 8 per-core input maps."""
    hs = np.ascontiguousarray(np.asarray(hidden_states, np.float32)[0])
    idx = np.asarray(idx).astype(np.int64)
    valid = np.asarray(valid).astype(bool)

    dt_proj, dt_wo, dt_wt = _np_dt(DT_PROJ), _np_dt(DT_WO), _np_dt(DT_WT)

    hsT = np.ascontiguousarray(hs.T).astype(dt_proj)       # [HID, S]

    srange = np.arange(S)
    cmask = ((idx <= srange[:, None]) & valid).ravel()
    flat = (idx * S + srange[:, None]).ravel()[cmask]
    eg = np.exp(np.asarray(geo_bias, np.float64))          # [H, S, K]

    in_maps = []
    for c in range(NCORES):
        h0 = HPC * c
        sl = slice(h0 * D, (h0 + HPC) * D)
        wt_c = np.empty((HPC, S, S), dt_wt)
        for hh in range(HPC):
            wt_c[hh] = (np.bincount(flat,
                                    weights=eg[h0 + hh].ravel()[cmask],
                                    minlength=S * S)
                        .reshape(S, S).astype(dt_wt))
        in_maps.append({
            "hsT": hsT,
            "wqT": np.ascontiguousarray(np.asarray(Wq)[sl].T).astype(dt_proj),
            "wkT": np.ascontiguousarray(np.asarray(Wk)[sl].T).astype(dt_proj),
            "wvT": np.ascontiguousarray(np.asarray(Wv)[sl].T).astype(dt_proj),
            "woT": np.ascontiguousarray(np.asarray(Wo)[:, sl].T).astype(dt_wo),
            "wt": wt_c,
        })
    return in_maps


def kernel(hidden_states, idx, valid, geo_bias, Wq, Wk, Wv, Wo, bo):
    from concourse import bass_utils

    nc = _get_nc()
    in_maps = make_in_maps(hidden_states, idx, valid, geo_bias, Wq, Wk, Wv, Wo)
    res = bass_utils.run_bass_kernel_spmd(nc, in_maps,
                                          core_ids=list(range(NCORES)))
    out = np.zeros((S, HID), np.float32)
    for r in res.results:
        out += r["y"].astype(np.float32)
    out += np.asarray(bo, np.float32)
    return out.reshape(B, S, HID)


# revision 13
# speedup vs baseline: 7.9652x; 7.4293x over previous
"""Sparse-attention Trainium2 kernel (nn_Attention_44341242364527).

Strategy
--------
Head-tensor-parallel over 8 NeuronCores (2 heads/core, Megatron-style:
Wq/Wk/Wv column-sharded, Wo row-sharded, partial outputs all-reduced on
the host during unshard).

The sparse gather ``k[idx]`` / ``v[idx]`` is reformulated densely: since
``exp(qk/sqrt(D) + geo) = exp(qk/sqrt(D)) * exp(geo)``, and idx/valid/
geo_bias are host-known inputs, the host pre-scatters

    WT[h][s', s] = sum_k 1[idx[s,k]==s' & valid & s'<=s] * exp(geo[h,s,k])

Then per head, on device (everything transposed so no on-chip transposes
are needed):

    ST  = Kh @ Qh.T                  [s', s]   (dense scores)
    AT  = exp(ST/sqrt(D)) * WT       [s', s]   (un-normalized attention)
    AOT = Vh.T @ AT                  [d, s]    (un-normalized context)
    Z   = colsum(AT)                 [1, s]    (softmax denominator)
    Y  += (AOT/Z).T @ WoT_shard      [s, HID]  (partial output)

Causality makes AT block-lower-triangular: only ~62% of blocks are
computed. WT==0 kills both the masked and the un-selected entries.

v2 changes vs the 193us baseline:
  - Z (the column sum of AT over s') moved off the PE: the AT tiles are
    tree-accumulated on the DVE (f32, SBUF 2x mode) into acc[128,512],
    then ONE GPSIMD partition_all_reduce per (j,head) broadcasts the
    128-partition sum to every partition. The broadcast output also
    replaces the ones-outer-product matmul (and its ACT copy) that used
    to materialize 1/Z across partitions: reciprocal runs directly on
    the broadcast. Saves ~39k PE cycles (~16us) per rep.
  - All matmul operands bf16 (was float32r): kills the fp32r 4x
    cycles/row penalty on <256-wide diagonal tiles, enables FWL on
    stationary loads and 2x/4x DVE modes on the exp*WT multiply.
  - PSUM rebalance: freed Z banks -> psX 4 bufs, psY 2 bufs (output
    projection double-buffered).
  - DMA batching: WT loaded in [128, 4*512] groups via a strided
    (t p) c -> p t c access pattern (20 descriptors/rep instead of 80);
    y written one row-block [128, 2048] at a time (16 instead of 64).
"""

import math
import sys

sys.path.insert(0, "/opt/trn_rl_repo")

import numpy as np

B, S, H, D, KS = 1, 2048, 16, 128, 64
HID = H * D
NCORES = 8
HPC = H // NCORES          # heads per core
CPC = HPC * D              # output channels per core
P = 128                    # partitions
SC = 512                   # s-chunk (PSUM bank width in f32)
NJ = S // SC               # 4 s-chunks
NT = S // P                # 16 s'-tiles
NK = HID // P              # 16 contraction chunks

# dtype knobs (numpy dtype name per tensor class); PSUM is always f32.
DT_PROJ = "bfloat16"       # hsT + Wq/Wk/Wv operands of the QKV projections
DT_QK = "bfloat16"         # Q^T/K^T operands of the score matmul
DT_ATT = "bfloat16"        # exp(S)*W and V operands of the AV matmul
DT_WT = "bfloat16"         # the scattered exp(geo) tensor (DMA-heavy)
DT_WO = "bfloat16"         # AOT and Wo operands of the output projection

_CACHE = {}


def _np_dt(name):
    if name == "bfloat16":
        import ml_dtypes

        return np.dtype(ml_dtypes.bfloat16)
    if name == "float32r":
        return np.dtype(np.float32)
    return np.dtype(name)


def _my_dt(name):
    from concourse import mybir

    return {
        "float32": mybir.dt.float32,
        "float32r": mybir.dt.float32r,
        "bfloat16": mybir.dt.bfloat16,
    }[name]


def _build_nc(reps=1):
    import concourse.tile as tile
    from concourse import bacc, bass_isa, mybir

    F32 = mybir.dt.float32
    EXP = mybir.ActivationFunctionType.Exp
    MULT = mybir.AluOpType.mult
    ADD = mybir.AluOpType.add
    RADD = bass_isa.ReduceOp.add

    nc = bacc.Bacc("TRN2", target_bir_lowering=False, debug=False,
                   num_devices=NCORES)

    hsT = nc.dram_tensor("hsT", [HID, S], _my_dt(DT_PROJ), kind="ExternalInput")
    wqT = nc.dram_tensor("wqT", [HID, CPC], _my_dt(DT_PROJ), kind="ExternalInput")
    wkT = nc.dram_tensor("wkT", [HID, CPC], _my_dt(DT_PROJ), kind="ExternalInput")
    wvT = nc.dram_tensor("wvT", [HID, CPC], _my_dt(DT_PROJ), kind="ExternalInput")
    woT = nc.dram_tensor("woT", [CPC, HID], _my_dt(DT_WO), kind="ExternalInput")
    wt = nc.dram_tensor("wt", [HPC, S, S], _my_dt(DT_WT), kind="ExternalInput")
    y = nc.dram_tensor("y", [S, HID], mybir.dt.bfloat16, kind="ExternalOutput")

    inv_sqrt_d = 1.0 / math.sqrt(D)

    def mm(out, lhsT, rhs, **kw):
        nc.tensor.matmul(out, lhsT, rhs, **kw)

    with tile.TileContext(nc) as tc, \
            nc.allow_low_precision(reason="bf16 matmul operands; PSUM accum stays f32"):
        with tc.tile_pool(name="persist", bufs=1) as persist:
            QT = [persist.tile([P, S], _my_dt(DT_QK), tag=f"qt{h}", name=f"qt{h}")
                  for h in range(HPC)]
            KT = [persist.tile([P, S], _my_dt(DT_QK), tag=f"kt{h}", name=f"kt{h}")
                  for h in range(HPC)]
            Vsb = [persist.tile([P, CPC], _my_dt(DT_ATT), tag=f"v{t}", name=f"vres{t}")
                   for t in range(NT)]
            AOT = [persist.tile([P, S], _my_dt(DT_WO), tag=f"aot{h}", name=f"aot{h}")
                   for h in range(HPC)]

            # Pipelined over j-chunks of the query/sequence dim: for each j,
            # project chunk j, run attention for chunk j (both heads), then
            # the output projection for s-tiles 4j..4j+3. Chunk-level deps
            # let the Tile scheduler overlap all three stages across j.
            with tc.tile_pool(name="wpool", bufs=1) as wpool, \
                 tc.tile_pool(name="hpool", bufs=20) as hpool, \
                 tc.tile_pool(name="wop", bufs=1) as wop, \
                 tc.tile_pool(name="wtp", bufs=5) as wtp, \
                 tc.tile_pool(name="atp", bufs=5) as atp, \
                 tc.tile_pool(name="rbp", bufs=4) as rbp, \
                 tc.tile_pool(name="zacc", bufs=4) as zaccp, \
                 tc.tile_pool(name="ypool", bufs=3) as ypool, \
                 tc.tile_pool(name="psP", bufs=2, space="PSUM") as psP, \
                 tc.tile_pool(name="psS", bufs=2, space="PSUM") as psS, \
                 tc.tile_pool(name="psA", bufs=2, space="PSUM") as psA, \
                 tc.tile_pool(name="psY", bufs=2, space="PSUM") as psY:
                wq_sb, wk_sb, wv_sb = [], [], []
                wo_sb = []

                for _rep in range(reps):
                    for j in range(NJ):
                        # -- QKV projection for chunk j --
                        # (weight loads interleaved k-wise with the first
                        # chunk's hsT loads so PE starts ~1us in, not after
                        # the full weight DMA)
                        # DMA order matters for the cold start: the first
                        # Q-projection chain needs all wq chunks + all hst
                        # chunks of j=0, so those stream first; wk/wv/wo
                        # follow (they are consumed by later chains).
                        hs_t = []
                        for k in range(NK):
                            if _rep == 0 and j == 0:
                                t_ = wpool.tile([P, CPC], _my_dt(DT_PROJ),
                                                tag=f"wq{k}", name=f"wq{k}")
                                nc.sync.dma_start(t_[:], wqT[k * P:(k + 1) * P, :])
                                wq_sb.append(t_)
                            t_ = hpool.tile([P, SC], _my_dt(DT_PROJ), tag="hst", name="hst")
                            nc.sync.dma_start(
                                t_[:], hsT[k * P:(k + 1) * P, j * SC:(j + 1) * SC])
                            hs_t.append(t_)
                        if _rep == 0 and j == 0:
                            for lst, dram, nm in ((wk_sb, wkT, "wk"),
                                                  (wv_sb, wvT, "wv")):
                                for k in range(NK):
                                    t_ = wpool.tile([P, CPC], _my_dt(DT_PROJ),
                                                    tag=f"{nm}{k}", name=f"{nm}{k}")
                                    nc.sync.dma_start(
                                        t_[:], dram[k * P:(k + 1) * P, :])
                                    lst.append(t_)
                            for h in range(HPC):
                                t_ = wop.tile([P, HID], _my_dt(DT_WO),
                                              tag=f"wo{h}", name=f"wo{h}")
                                nc.sync.dma_start(t_[:], woT[h * P:(h + 1) * P, :])
                                wo_sb.append(t_)
                        for h in range(HPC):
                            for w_sb, acc in ((wq_sb, QT), (wk_sb, KT)):
                                pp = psP.tile([P, SC], F32, tag="big", name="ps_proj")
                                for k in range(NK):
                                    mm(pp[:], w_sb[k][:, h * D:(h + 1) * D],
                                       hs_t[k][:],
                                       start=(k == 0), stop=(k == NK - 1))
                                nc.scalar.copy(
                                    acc[h][:, j * SC:(j + 1) * SC], pp[:])
                        for si in range(SC // P):
                            vp = psP.tile([P, CPC], F32, tag="big", name="ps_projv")
                            for k in range(NK):
                                mm(vp[:], hs_t[k][:, si * P:(si + 1) * P],
                                   wv_sb[k][:],
                                   start=(k == 0), stop=(k == NK - 1))
                            nc.vector.tensor_copy(Vsb[4 * j + si][:], vp[:])

                        # -- attention for chunk j: both heads interleaved,
                        # AV matmuls lag the ST/exp/mul stage by 2 items so
                        # the PE never waits on the ACT->DVE latency chain.
                        # Z runs off-PE: DVE tree-accumulates the AT tiles
                        # into acc[h], then one GPSIMD partition_all_reduce
                        # broadcasts the column sum to all partitions. --
                        tmax = min(4 * j + 3, NT - 1)
                        aop = [psA.tile([P, SC], F32, tag="ao", name=f"ao{h}")
                               for h in range(HPC)]
                        zacc = [zaccp.tile([P, SC], F32, tag="za", name=f"za{h}")
                                for h in range(HPC)]
                        wtg = {}
                        for h in range(HPC):
                            for g in range(0, tmax + 1, 4):
                                t_ = wtp.tile([P, 4 * SC], _my_dt(DT_WT),
                                              tag="wt", name="wt")
                                dram_v = wt[h].rearrange(
                                    "(t p) c -> p t c", p=P)[
                                        :, g:g + 4, j * SC:(j + 1) * SC]
                                sb_v = t_[:].rearrange("p (t c) -> p t c", c=SC)
                                nc.sync.dma_start(sb_v, dram_v)
                                wtg[(h, g)] = t_
                        items = [(t, h) for t in range(tmax + 1)
                                 for h in range(HPC)]
                        pend = []

                        def drain_one():
                            t_, h_, at_, o_, w_ = pend.pop(0)
                            mm(aop[h_][:, o_:SC],
                               Vsb[t_][:, h_ * D:(h_ + 1) * D],
                               at_[:, :w_],
                               start=(t_ == 0), stop=(t_ == tmax))

                        for t, h in items:
                            # within the diagonal block only columns
                            # s >= 128t are causally reachable (WT is zero
                            # elsewhere) -- shrink every stage to that width
                            o = max(0, t * P - j * SC)
                            w = SC - o
                            stp = psS.tile([P, SC], F32, tag="st", name="st")
                            mm(stp[:, :w], KT[h][:, t * P:(t + 1) * P],
                               QT[h][:, j * SC + o:(j + 1) * SC],
                               start=True, stop=True)
                            at = atp.tile([P, SC], _my_dt(DT_ATT), tag="at", name="at")
                            nc.scalar.activation(at[:, :w], stp[:, :w], EXP,
                                                 scale=inv_sqrt_d)
                            g = (t // 4) * 4
                            base = (t - g) * SC
                            nc.vector.tensor_mul(at[:, :w], at[:, :w],
                                                 wtg[(h, g)][:, base + o:base + SC])
                            if t == 0:
                                nc.vector.tensor_copy(zacc[h][:], at[:])
                            else:
                                nc.vector.tensor_tensor(
                                    zacc[h][:, o:SC], zacc[h][:, o:SC],
                                    at[:, :w], ADD)
                            pend.append((t, h, at, o, w))
                            if len(pend) >= 3:
                                drain_one()
                        while pend:
                            drain_one()

                        for h in range(HPC):
                            zb = rbp.tile([P, SC], F32, tag="zb", name="zb")
                            nc.gpsimd.partition_all_reduce(
                                zb[:], zacc[h][:], channels=P, reduce_op=RADD)
                            rbs = rbp.tile([P, SC], F32, tag="rbs", name="rbs")
                            nc.vector.reciprocal(rbs[:], zb[:])
                            nc.vector.tensor_tensor(
                                AOT[h][:, j * SC:(j + 1) * SC], aop[h][:],
                                rbs[:], MULT)

                        # -- output projection for s-tiles of chunk j --
                        for m in range(4 * j, 4 * j + 4):
                            ysb = ypool.tile([P, S], mybir.dt.bfloat16,
                                             tag="ysb", name="ysb")
                            for n in range(NJ):
                                yps = psY.tile([P, SC], F32, tag="y", name="ps_y")
                                for h in range(HPC):
                                    mm(yps[:], AOT[h][:, m * P:(m + 1) * P],
                                       wo_sb[h][:, n * SC:(n + 1) * SC],
                                       start=(h == 0), stop=(h == HPC - 1))
                                nc.scalar.copy(
                                    ysb[:, n * SC:(n + 1) * SC], yps[:])
                            nc.sync.dma_start(y[m * P:(m + 1) * P, :], ysb[:])

    nc.compile()
    return nc


def _get_nc():
    if "nc" not in _CACHE:
        _CACHE["nc"] = _build_nc()
    return _CACHE["nc"]


def make_in_maps(hidden_states, idx, valid, geo_bias, Wq, Wk, Wv, Wo):
    """Host-side sharding/layout prep: returns the 8 per-core input maps."""
    hs = np.ascontiguousarray(np.asarray(hidden_states, np.float32)[0])
    idx = np.asarray(idx).astype(np.int64)
    valid = np.asarray(valid).astype(bool)

    dt_proj, dt_wo, dt_wt = _np_dt(DT_PROJ), _np_dt(DT_WO), _np_dt(DT_WT)

    hsT = np.ascontiguousarray(hs.T).astype(dt_proj)       # [HID, S]

    srange = np.arange(S)
    cmask = ((idx <= srange[:, None]) & valid).ravel()
    flat = (idx * S + srange[:, None]).ravel()[cmask]
    eg = np.exp(np.asarray(geo_bias, np.float64))          # [H, S, K]

    in_maps = []
    for c in range(NCORES):
        h0 = HPC * c
        sl = slice(h0 * D, (h0 + HPC) * D)
        wt_c = np.empty((HPC, S, S), dt_wt)
        for hh in range(HPC):
            wt_c[hh] = (np.bincount(flat,
                                    weights=eg[h0 + hh].ravel()[cmask],
                                    minlength=S * S)
                        .reshape(S, S).astype(dt_wt))
        in_maps.append({
            "hsT": hsT,
            "wqT": np.ascontiguousarray(np.asarray(Wq)[sl].T).astype(dt_proj),
            "wkT": np.ascontiguousarray(np.asarray(Wk)[sl].T).astype(dt_proj),
            "wvT": np.ascontiguousarray(np.asarray(Wv)[sl].T).astype(dt_proj),
            "woT": np.ascontiguousarray(np.asarray(Wo)[:, sl].T).astype(dt_wo),
            "wt": wt_c,
        })
    return in_maps


def kernel(hidden_states, idx, valid, geo_bias, Wq, Wk, Wv, Wo, bo):
    from concourse import bass_utils

    nc = _get_nc()
    in_maps = make_in_maps(hidden_states, idx, valid, geo_bias, Wq, Wk, Wv, Wo)
    res = bass_utils.run_bass_kernel_spmd(nc, in_maps,
                                          core_ids=list(range(NCORES)))
    out = np.zeros((S, HID), np.float32)
    for r in res.results:
        out += r["y"].astype(np.float32)
    out += np.asarray(bo, np.float32)
    return out.reshape(B, S, HID)
